# revision 5
# baseline (speedup 1.0000x reference)
"""Trainium2 Bass kernel for nn_DecoderLayer (moe_routing), 8 NeuronCores.

Decomposition (expert-parallel MoE + token-parallel attention):

  kernel A (SPMD, core = (batch b, half c)): each core owns 512 queries of one
    batch (64-row interleave so causal work is balanced and the program is
    identical across cores).  All matmul data is bf16 (PE runs 1 cyc/row vs 4
    for fp32); the f32 residual stream and f32 xhat3 keep accuracy.  CA K/V
    projections (which depend only on src) are issued FIRST so the PE stays
    busy during LN phases and the HAM clock gate keeps the PE at 2.4 GHz.
    LN1 -> self-attn -> LN2 -> cross-attn -> LN3.  Attention runs in S^T
    (keys-on-partitions) layout with softmax denominators from an appended
    ones-column of V; normalization is fused into the PSUM->SBUF drain.
    K biases are dropped entirely (softmax-invariant); V/out biases are
    folded into the residual input (host) or one bias matmul (CA).

  host: router logits from f32 xhat3 (f32 routing avoids bf16 argmax flips),
    softmax/argmax, capacity-bucketed all-to-all token dispatch.

  kernel B (SPMD, core = expert e): y = relu(x @ w1[e] + b1[e]) @ w2[e] + b2[e]
    over the CAP-padded token batch routed to that expert.  Weights stream in
    per-block on the SP HWDGE queue so compute starts ~2us in instead of
    waiting 26us for the monolithic loads.

  host: gate * token_mask scaling, scatter back, residual add.
"""

import numpy as np
import ml_dtypes

import concourse.bacc as bacc
import concourse.bass as bass
import concourse.tile as tile
from concourse import mybir
from concourse.bass_utils import run_bass_kernel_spmd
from concourse.masks import make_identity

B, T, S, D, H, E, FF = 4, 1024, 1024, 512, 8, 8, 2048
HD = D // H
P = 128
NKT = T // P          # 8 key tiles
NQ = 512              # queries per core
DCH = D // P          # 4 feature chunks
FCH = FF // P         # 16 FF chunks
CAP = 640             # expert capacity (max observed count 559)
NCAP = CAP // 2       # kernel-B moving-dim chunk (320)
NEG = -1e9
F32 = mybir.dt.float32
BF16 = mybir.dt.bfloat16

_cache = {}

# These track the most recent run for test harnesses.
last_exec_ns = {}


# --------------------------------------------------------------------------
# kernel A builder
# --------------------------------------------------------------------------

def _attention(nc, wp, tp, ps, KT_sb, QT_sb, V_sb, attnoutT_sb,
               pad_sb, dmask_sb, causal, tag):
    """S^T-layout attention: fills attnoutT_sb [128, DCH, NQ] (normalized)."""
    onehot = wp["onehot"]
    denoms = tp.tile([E, NQ], F32, tag="denoms", bufs=1, name=f"denoms_{tag}")
    recips = tp.tile([E, NQ], BF16, tag="recips", bufs=1, name=f"recips_{tag}")
    for h in range(H):
        av = ps.tile([HD + 1, NQ], F32, tag="av", bufs=2, name=f"av{h}_{tag}")
        for kc in range(NKT):
            n0 = 64 * kc if causal else 0
            n = NQ - n0
            st = ps.tile([P, NQ], F32, tag="big", bufs=4, name=f"st{h}_{kc}_{tag}")
            nc.tensor.matmul(
                st[:, 0:n],
                KT_sb[(h % 2) * HD:(h % 2) * HD + HD, h // 2, kc * P:(kc + 1) * P],
                QT_sb[(h % 2) * HD:(h % 2) * HD + HD, h // 2, n0:NQ],
                start=True, stop=True,
            )
            if causal:
                nc.vector.tensor_tensor(
                    st[:, 0:64], st[:, 0:64], dmask_sb[:, kc, :],
                    op=mybir.AluOpType.add,
                )
            pt = tp.tile([P, NQ], BF16, tag="pt", bufs=4, name=f"pt{h}_{kc}_{tag}")
            nc.scalar.activation(
                pt[:, 0:n], st[:, 0:n], mybir.ActivationFunctionType.Exp,
                bias=pad_sb[:, kc:kc + 1], scale=0.125,
            )
            nc.tensor.matmul(
                av[:, n0:NQ],
                V_sb[:, kc, h, 0:HD + 1],
                pt[:, 0:n],
                start=(kc == 0), stop=(kc == NKT - 1),
                skip_group_check=True,
            )
        dstage = tp.tile([1, NQ], F32, tag="dstage", bufs=4, name=f"dst{h}_{tag}")
        nc.vector.tensor_copy(dstage[:, :], av[HD:HD + 1, :])
        nc.sync.dma_start(denoms[h:h + 1, :], dstage[:, :])
        nc.vector.tensor_copy(attnoutT_sb[(h % 2) * HD:(h % 2) * HD + HD,
                                          h // 2, :], av[0:HD, :])
    with nc.allow_low_precision(reason="bf16 recips feed a bf16 matmul"):
        nc.vector.reciprocal(recips[:, :], denoms[:, :])
    for h in range(H):
        po = (h % 2) * HD
        bc = ps.tile([HD, NQ], F32, tag="bc", bufs=1, name=f"bc{h}_{tag}")
        nc.tensor.matmul(bc[:, :], onehot[:, h * HD:(h + 1) * HD], recips[:, :],
                         start=True, stop=True)
        nc.vector.tensor_tensor(
            attnoutT_sb[po:po + HD, h // 2, :],
            attnoutT_sb[po:po + HD, h // 2, :], bc[:, :],
            op=mybir.AluOpType.mult,
        )


def _ln_tiles(nc, wp, tp, src_ap_list, dma_out, xT_sb, ps, identity, tag):
    """LayerNorm per 128-row tile (batched by op kind so the ACT table set
    isn't reloaded per tile).  If xT_sb is given, the normalized tiles are
    written bf16 and transposed into it; if dma_out is given, they are
    written f32 straight to DRAM (no transpose)."""
    eps = wp["eps"]
    nt = len(src_ap_list)
    mvs, rstds, nmrs = [], [], []
    for i, x_ap in enumerate(src_ap_list):
        stats = tp.tile([P, 6], F32, tag="stats", name=f"stats{i}_{tag}")
        mv = tp.tile([P, 2], F32, tag="mv", bufs=8, name=f"mv{i}_{tag}")
        nc.vector.bn_stats(stats[:, :], x_ap)
        nc.vector.bn_aggr(mv[:, :], stats[:, :])
        mvs.append(mv)
    for i in range(nt):
        rstd = tp.tile([P, 1], F32, tag="rstd", bufs=8, name=f"rstd{i}_{tag}")
        nc.scalar.activation(rstd[:, :], mvs[i][:, 1:2],
                             mybir.ActivationFunctionType.Ln, bias=eps[:, :])
        rstds.append(rstd)
    for i in range(nt):
        nc.scalar.activation(rstds[i][:, :], rstds[i][:, :],
                             mybir.ActivationFunctionType.Exp, scale=-0.5)
    for i in range(nt):
        nmr = tp.tile([P, 1], F32, tag="nmr", bufs=8, name=f"nmr{i}_{tag}")
        nc.vector.tensor_scalar(nmr[:, :], mvs[i][:, 0:1], rstds[i][:, :], -1.0,
                                op0=mybir.AluOpType.mult,
                                op1=mybir.AluOpType.mult)
        nmrs.append(nmr)
    for i, x_ap in enumerate(src_ap_list):
        if dma_out is not None:
            xh = tp.tile([P, D], F32, tag="xh32", bufs=2, name=f"xh32_{i}_{tag}")
            nc.scalar.activation(xh[:, :], x_ap,
                                 mybir.ActivationFunctionType.Identity,
                                 bias=nmrs[i][:, :], scale=rstds[i][:, :])
            nc.scalar.dma_start(dma_out[i], xh[:, :])
        if xT_sb is not None:
            xhb = tp.tile([P, D], BF16, tag="xh", bufs=3, name=f"xh{i}_{tag}")
            nc.scalar.activation(xhb[:, :], x_ap,
                                 mybir.ActivationFunctionType.Identity,
                                 bias=nmrs[i][:, :], scale=rstds[i][:, :])
            for dch in range(DCH):
                tr = ps.tile([P, P], BF16, tag="tr", bufs=1,
                             name=f"tr{i}_{dch}_{tag}")
                nc.tensor.transpose(tr[:, :], xhb[:, dch * P:(dch + 1) * P],
                                    identity)
                nc.vector.tensor_copy(xT_sb[:, dch, i * P:(i + 1) * P], tr[:, :])


def build_kernel_a():
    nc = bacc.Bacc(None, target_bir_lowering=False)

    def din(name, shape, dt=F32):
        return nc.dram_tensor(name, shape, dt, kind="ExternalInput")

    tgt_rolled = din("tgt_rolled", [T, D])
    tgt_q = din("tgt_q", [NQ, D])          # host-folded: tgt[qidx] + sa_bo_eff
    srcT = din("srcT", [D, S], BF16)
    sa_winT = din("sa_winT", [D, 3 * D], BF16)
    sa_bq = din("sa_bq", [P, 4])
    sa_woT = din("sa_woT", [D, D], BF16)
    ca_winT = din("ca_winT", [D, 3 * D], BF16)
    ca_bq = din("ca_bq", [P, 4])
    ca_woT = din("ca_woT", [D, D], BF16)
    ca_bo = din("ca_bo", [1, D], BF16)     # host-folded: ca_bo + ca_bv @ ca_wo
    onehot_d = din("onehot", [E, D], BF16)
    dmask = din("dmask", [P, NKT, 64])
    sa_pad = din("sa_pad", [P, NKT])
    ca_pad = din("ca_pad", [P, NKT])

    tgt2_d = nc.dram_tensor("tgt2", [NQ, D], F32, kind="ExternalOutput")
    xhat3_d = nc.dram_tensor("xhat3", [NQ, D], F32, kind="ExternalOutput")

    with tile.TileContext(nc) as tc:
        with (
            tc.tile_pool(name="wpool", bufs=1) as wpool,
            tc.tile_pool(name="apool", bufs=1) as apool,
            tc.tile_pool(name="tpool", bufs=2) as tpool,
            tc.tile_pool(name="pspool", bufs=1, space="PSUM") as pspool,
        ):
            # ---- load constants/weights (SP HWDGE queue; CA-first order) ----
            def wload(name, ap_dram, shape, rearr=None, dt=F32):
                t = wpool.tile(shape, dt, name=name)
                src = ap_dram[:] if rearr is None else ap_dram.rearrange(rearr, p=P)
                nc.sync.dma_start(t[:], src)
                return t

            w = {}
            srcT_sb = apool.tile([P, DCH, S], BF16, name="srcT_sb")
            nc.sync.dma_start(srcT_sb[:], srcT.rearrange("(c p) n -> p c n", p=P))
            w["ca_winT"] = wload("ca_winT_t", ca_winT, [P, DCH, 3 * D],
                                 "(c p) n -> p c n", dt=BF16)
            w["sa_winT"] = wload("sa_winT_t", sa_winT, [P, DCH, 3 * D],
                                 "(c p) n -> p c n", dt=BF16)
            w["sa_woT"] = wload("sa_woT_t", sa_woT, [P, DCH, D],
                                "(c p) n -> p c n", dt=BF16)
            w["ca_woT"] = wload("ca_woT_t", ca_woT, [P, DCH, D],
                                "(c p) n -> p c n", dt=BF16)
            w["sa_bq"] = wload("sa_bq_t", sa_bq, [P, 4])
            w["ca_bq"] = wload("ca_bq_t", ca_bq, [P, 4])
            w["ca_bo"] = wload("ca_bo_t", ca_bo, [1, D], dt=BF16)
            w["dmask"] = wload("dmask_t", dmask, [P, NKT, 64])
            w["sa_pad"] = wload("sa_pad_t", sa_pad, [P, NKT])
            w["ca_pad"] = wload("ca_pad_t", ca_pad, [P, NKT])
            onehot = wpool.tile([E, D], BF16, name="onehot")
            nc.sync.dma_start(onehot[:], onehot_d[:])
            w["onehot"] = onehot

            identity = wpool.tile([P, P], BF16, name="identity")
            make_identity(nc, identity)
            ones1 = wpool.tile([1, P], BF16, name="ones1")
            nc.vector.memset(ones1[:, :], 1.0)
            eps = wpool.tile([P, 1], F32, name="eps")
            nc.vector.memset(eps[:, :], 1e-5)
            w["ones1"] = ones1
            w["eps"] = eps

            # ---- activation/residual DMAs (gpsimd SWDGE queue) ----
            x_tiles = []
            for i in range(NKT):
                xt = tpool.tile([P, D], F32, tag="xin", bufs=8, name=f"xin{i}")
                nc.gpsimd.dma_start(xt[:], tgt_rolled[i * P:(i + 1) * P, :])
                x_tiles.append(xt[:, :])
            tq_tiles = []
            for qt in range(DCH):
                tq = tpool.tile([P, D], F32, tag="tgtq", bufs=4, name=f"tq{qt}")
                nc.gpsimd.dma_start(tq[:], tgt_q[qt * P:(qt + 1) * P, :])
                tq_tiles.append(tq)

            # persistent activation tensors
            xT_sb = apool.tile([P, DCH, T], BF16, name="xT_sb")
            KT_sb = apool.tile([P, DCH, T], BF16, name="KT_sb")
            KT2_sb = apool.tile([P, DCH, T], BF16, name="KT2_sb")
            QT_sb = apool.tile([P, DCH, NQ], BF16, name="QT_sb")
            V_sb = apool.tile([P, NKT, H, HD + 1], BF16, name="V_sb")
            V2_sb = apool.tile([P, NKT, H, HD + 1], BF16, name="V2_sb")
            attnoutT_sb = apool.tile([P, DCH, NQ], BF16, name="attnoutT_sb")
            tgt1_sb = apool.tile([P, DCH, D], F32, name="tgt1_sb")

            nc.vector.memset(V_sb[:, :, :, HD:HD + 1], 1.0)
            nc.vector.memset(V2_sb[:, :, :, HD:HD + 1], 1.0)

            # ---- EARLY: CA K/V projections (depend only on srcT) ----
            # keeps the PE busy while LN1 runs on Vector/Scalar
            for m in range(DCH):  # K from srcT; no K bias (softmax-invariant)
                for nch in range(2):
                    pp = pspool.tile([P, 512], F32, tag="big", bufs=4,
                                     name=f"ck{m}_{nch}")
                    for dch in range(DCH):
                        nc.tensor.matmul(
                            pp[:, :],
                            w["ca_winT"][:, dch, D + m * P:D + (m + 1) * P],
                            srcT_sb[:, dch, nch * 512:(nch + 1) * 512],
                            start=(dch == 0), stop=(dch == DCH - 1),
                        )
                    nc.vector.tensor_copy(
                        KT2_sb[:, m, nch * 512:(nch + 1) * 512], pp[:, :])
            for kt in range(NKT):  # V from srcT; V bias folded into out bias
                pp = pspool.tile([P, D], F32, tag="big", bufs=4, name=f"cv{kt}")
                for dch in range(DCH):
                    nc.tensor.matmul(
                        pp[:, :],
                        srcT_sb[:, dch, kt * P:(kt + 1) * P],
                        w["ca_winT"][:, dch, 2 * D:3 * D],
                        start=(dch == 0), stop=(dch == DCH - 1),
                    )
                nc.vector.tensor_copy(
                    V2_sb[:, kt, :, 0:HD],
                    pp[:, :].rearrange("p (h e) -> p h e", e=HD))

            # ---- LN1 over rolled batch + transpose ----
            _ln_tiles(nc, w, tpool, x_tiles, None, xT_sb, pspool, identity,
                      tag="ln1")

            # ---- SA projections ----
            for m in range(DCH):  # K (no bias)
                for nch in range(2):
                    pp = pspool.tile([P, 512], F32, tag="big", bufs=4,
                                     name=f"pk{m}_{nch}")
                    for dch in range(DCH):
                        nc.tensor.matmul(
                            pp[:, :],
                            w["sa_winT"][:, dch, D + m * P:D + (m + 1) * P],
                            xT_sb[:, dch, nch * 512:(nch + 1) * 512],
                            start=(dch == 0), stop=(dch == DCH - 1),
                        )
                    nc.vector.tensor_copy(
                        KT_sb[:, m, nch * 512:(nch + 1) * 512], pp[:, :])
            # Q (own queries = first 64 cols of each 128-block of xT)
            q_rhs = [xT_sb[:, dch, :].rearrange("p (b c) -> p b c", c=P)[:, :, 0:64]
                     for dch in range(DCH)]
            for m in range(DCH):
                pp = pspool.tile([P, NQ], F32, tag="big", bufs=4, name=f"pq{m}")
                for dch in range(DCH):
                    nc.tensor.matmul(
                        pp[:, :].rearrange("p (b c) -> p b c", c=64),
                        w["sa_winT"][:, dch, m * P:(m + 1) * P],
                        q_rhs[dch],
                        start=(dch == 0), stop=(dch == DCH - 1),
                    )
                nc.scalar.activation(
                    QT_sb[:, m, :], pp[:, :],
                    mybir.ActivationFunctionType.Identity,
                    bias=w["sa_bq"][:, m:m + 1])
            for kt in range(NKT):  # V (bias folded)
                pp = pspool.tile([P, D], F32, tag="big", bufs=4, name=f"pv{kt}")
                for dch in range(DCH):
                    nc.tensor.matmul(
                        pp[:, :],
                        xT_sb[:, dch, kt * P:(kt + 1) * P],
                        w["sa_winT"][:, dch, 2 * D:3 * D],
                        start=(dch == 0), stop=(dch == DCH - 1),
                    )
                nc.vector.tensor_copy(
                    V_sb[:, kt, :, 0:HD],
                    pp[:, :].rearrange("p (h e) -> p h e", e=HD))

            # ---- SA attention ----
            _attention(nc, w, tpool, pspool, KT_sb, QT_sb, V_sb,
                       attnoutT_sb, w["sa_pad"], w["dmask"], causal=True,
                       tag="sa")

            # ---- SA out-proj + residual (out bias host-folded into tgt_q) ----
            for qt in range(DCH):
                pp = pspool.tile([P, D], F32, tag="big", bufs=4, name=f"po{qt}")
                for dch in range(DCH):
                    nc.tensor.matmul(
                        pp[:, :],
                        attnoutT_sb[:, dch, qt * P:(qt + 1) * P],
                        w["sa_woT"][:, dch, :],
                        start=(dch == 0), stop=(dch == DCH - 1))
                nc.vector.tensor_tensor(tgt1_sb[:, qt, :], pp[:, :],
                                        tq_tiles[qt][:, :],
                                        op=mybir.AluOpType.add)

            # ---- LN2 + transpose (reuse xT_sb cols 0:NQ) ----
            _ln_tiles(nc, w, tpool,
                      [tgt1_sb[:, i, :] for i in range(DCH)],
                      None, xT_sb, pspool, identity, tag="ln2")

            # ---- CA Q projection ----
            for m in range(DCH):
                pp = pspool.tile([P, NQ], F32, tag="big", bufs=4, name=f"cq{m}")
                for dch in range(DCH):
                    nc.tensor.matmul(
                        pp[:, :],
                        w["ca_winT"][:, dch, m * P:(m + 1) * P],
                        xT_sb[:, dch, 0:NQ],
                        start=(dch == 0), stop=(dch == DCH - 1),
                    )
                nc.scalar.activation(
                    QT_sb[:, m, :], pp[:, :],
                    mybir.ActivationFunctionType.Identity,
                    bias=w["ca_bq"][:, m:m + 1])

            # ---- CA attention ----
            _attention(nc, w, tpool, pspool, KT2_sb, QT_sb, V2_sb,
                       attnoutT_sb, w["ca_pad"], None, causal=False,
                       tag="ca")

            # ---- CA out-proj + bias + residual ----
            for qt in range(DCH):
                pp = pspool.tile([P, D], F32, tag="big", bufs=4, name=f"co{qt}")
                for dch in range(DCH):
                    nc.tensor.matmul(
                        pp[:, :],
                        attnoutT_sb[:, dch, qt * P:(qt + 1) * P],
                        w["ca_woT"][:, dch, :],
                        start=(dch == 0), stop=False)
                nc.tensor.matmul(pp[:, :], ones1[0:1, 0:P], w["ca_bo"][0:1, :],
                                 start=False, stop=True)
                nc.vector.tensor_tensor(tgt1_sb[:, qt, :], pp[:, :],
                                        tgt1_sb[:, qt, :],
                                        op=mybir.AluOpType.add)
            nc.gpsimd.dma_start(tgt2_d.rearrange("(a p) d -> p a d", p=P),
                                tgt1_sb[:])

            # ---- LN3 (xhat3 streamed straight to DRAM; no transpose) ----
            _ln_tiles(nc, w, tpool,
                      [tgt1_sb[:, i, :] for i in range(DCH)],
                      [xhat3_d[i * P:(i + 1) * P, :] for i in range(DCH)],
                      None, pspool, identity, tag="ln3")

    nc.compile()
    return nc


# --------------------------------------------------------------------------
# kernel B builder (one expert per core)
# --------------------------------------------------------------------------

def build_kernel_b():
    nc = bacc.Bacc(None, target_bir_lowering=False)
    x3T = nc.dram_tensor("x3T", [D, CAP], BF16, kind="ExternalInput")
    w1 = nc.dram_tensor("w1e", [D, FF], BF16, kind="ExternalInput")
    b1 = nc.dram_tensor("b1e", [P, FCH], F32, kind="ExternalInput")
    w2 = nc.dram_tensor("w2e", [FF, D], BF16, kind="ExternalInput")
    b2 = nc.dram_tensor("b2e", [P, DCH], F32, kind="ExternalInput")
    yT = nc.dram_tensor("yT", [D, CAP], F32, kind="ExternalOutput")

    with tile.TileContext(nc) as tc:
        with (
            tc.tile_pool(name="wp", bufs=1) as wp,
            tc.tile_pool(name="ap", bufs=1) as ap_,
            tc.tile_pool(name="ps", bufs=2, space="PSUM") as ps,
        ):
            # biases + first x chunk first (gpsimd queue)
            b1_sb = wp.tile([P, FCH], F32, name="b1_sb")
            nc.gpsimd.dma_start(b1_sb[:], b1[:])
            b2_sb = wp.tile([P, DCH], F32, name="b2_sb")
            nc.gpsimd.dma_start(b2_sb[:], b2[:])
            x3T_sb = ap_.tile([P, DCH, CAP], BF16, name="x3T_sb")
            nc.gpsimd.dma_start(
                x3T_sb[:, :, 0:NCAP],
                x3T[:, 0:NCAP].rearrange("(c p) n -> p c n", p=P))
            nc.gpsimd.dma_start(
                x3T_sb[:, :, NCAP:CAP],
                x3T[:, NCAP:CAP].rearrange("(c p) n -> p c n", p=P))

            # per-block weight streams (SP HWDGE queue): compute starts after
            # the first block instead of after the full 4 MB
            w1_blk = []
            for fm in range(FCH):
                t = wp.tile([P, DCH, P], BF16, name=f"w1_{fm}")
                nc.sync.dma_start(
                    t[:], w1[:, fm * P:(fm + 1) * P].rearrange(
                        "(c p) n -> p c n", p=P))
                w1_blk.append(t)
            w2_blk = []
            for dm in range(DCH):
                t = wp.tile([P, FCH, P], BF16, name=f"w2_{dm}")
                nc.sync.dma_start(
                    t[:], w2[:, dm * P:(dm + 1) * P].rearrange(
                        "(c p) n -> p c n", p=P))
                w2_blk.append(t)

            hT_sb = ap_.tile([P, FCH, CAP], BF16, name="hT_sb")
            for fm in range(FCH):
                for nch in range(CAP // NCAP):
                    ph = ps.tile([P, NCAP], F32, tag="ph", bufs=4,
                                 name=f"ph{fm}_{nch}")
                    for dch in range(DCH):
                        nc.tensor.matmul(
                            ph[:, :],
                            w1_blk[fm][:, dch, :],
                            x3T_sb[:, dch, nch * NCAP:(nch + 1) * NCAP],
                            start=(dch == 0), stop=(dch == DCH - 1),
                        )
                    if fm % 2 == 0:  # split relu epilogues across engines
                        nc.scalar.activation(
                            hT_sb[:, fm, nch * NCAP:(nch + 1) * NCAP], ph[:, :],
                            mybir.ActivationFunctionType.Relu,
                            bias=b1_sb[:, fm:fm + 1])
                    else:
                        nc.vector.tensor_scalar(
                            hT_sb[:, fm, nch * NCAP:(nch + 1) * NCAP], ph[:, :],
                            b1_sb[:, fm:fm + 1], 0.0,
                            op0=mybir.AluOpType.add,
                            op1=mybir.AluOpType.max)
            for dm in range(DCH):
                yT_sb = ap_.tile([P, CAP], F32, tag="yt", bufs=4,
                                 name=f"yT_sb{dm}")
                for nch in range(CAP // NCAP):
                    py = ps.tile([P, NCAP], F32, tag="py", bufs=4,
                                 name=f"py{dm}_{nch}")
                    for fch in range(FCH):
                        nc.tensor.matmul(
                            py[:, :],
                            w2_blk[dm][:, fch, :],
                            hT_sb[:, fch, nch * NCAP:(nch + 1) * NCAP],
                            start=(fch == 0), stop=(fch == FCH - 1),
                        )
                    nc.vector.tensor_scalar(
                        yT_sb[:, nch * NCAP:(nch + 1) * NCAP], py[:, :],
                        b2_sb[:, dm:dm + 1], None,
                        op0=mybir.AluOpType.add)
                nc.scalar.dma_start(
                    yT[dm * P:(dm + 1) * P, :], yT_sb[:])

    nc.compile()
    return nc


# --------------------------------------------------------------------------
# host orchestration
# --------------------------------------------------------------------------

def _onehot_blocks():
    oh = np.zeros((E, D), np.float32)
    for h in range(H):
        oh[h, h * HD:(h + 1) * HD] = 1.0
    return oh


def _host_prep(inputs):
    f32 = np.float32
    bf = ml_dtypes.bfloat16

    def a(k):
        return np.asarray(inputs[k]).astype(f32) if inputs[k] is not None else None

    g1, b1 = a("ln1_g"), a("ln1_b")
    g2, b2 = a("ln2_g"), a("ln2_b")
    g3, b3 = a("ln3_g"), a("ln3_b")
    sa_win, sa_bin = a("sa_win"), a("sa_bin")
    ca_win, ca_bin = a("ca_win"), a("ca_bin")

    sa_winf = sa_win * g1[None, :]
    sa_binf = sa_bin + sa_win @ b1
    ca_winf = ca_win.copy()
    ca_binf = ca_bin.copy()
    ca_winf[:D] = ca_win[:D] * g2[None, :]
    ca_binf[:D] = ca_bin[:D] + ca_win[:D] @ b2
    router_w = a("router_w")
    router_wf = router_w * g3[None, :]
    router_bf = a("router_b") + router_w @ b3
    w1_ = a("w1")
    w1f = w1_ * g3[None, :, None]
    b1f = a("b1") + np.einsum("d,edf->ef", b3, w1_)

    # V-bias and out-bias fold:  attn_norm @ Wo + bo == attn_noVbias @ Wo +
    # (bv @ Wo + bo)  because softmax weights sum to 1 per head.
    sa_bo_eff = a("sa_bo") + sa_binf[2 * D:] @ a("sa_wo").T
    ca_bo_eff = a("ca_bo") + ca_binf[2 * D:] @ a("ca_wo").T

    def chunks(v):  # [n] -> [128, n//128] chunk-major columns
        return np.ascontiguousarray(v.reshape(-1, P).T)

    prep = dict(
        sa_winT=np.ascontiguousarray(sa_winf.T).astype(bf),
        sa_bq=np.ascontiguousarray(sa_binf[:D].reshape(4, P).T),
        sa_woT=np.ascontiguousarray(a("sa_wo").T).astype(bf),
        ca_winT=np.ascontiguousarray(ca_winf.T).astype(bf),
        ca_bq=np.ascontiguousarray(ca_binf[:D].reshape(4, P).T),
        ca_woT=np.ascontiguousarray(a("ca_wo").T).astype(bf),
        ca_bo=np.ascontiguousarray(ca_bo_eff.reshape(1, D)).astype(bf),
        onehot=_onehot_blocks().astype(bf),
        router_wf=router_wf, router_bf=router_bf,
        w1f=w1f.astype(bf), b1c=np.stack([chunks(b1f[e]) for e in range(E)]),
        w2=a("w2").astype(bf), b2c=np.stack([chunks(a("b2")[e]) for e in range(E)]),
    )

    tgt, src = a("tgt"), a("src")
    tgt_mask = np.asarray(inputs["tgt_mask"])
    tgt_pad = np.asarray(inputs["tgt_pad_mask"])
    src_pad = np.asarray(inputs["src_pad_mask"])

    cores = []
    for b in range(B):
        srcTb = np.ascontiguousarray(src[b].T).astype(bf)
        for c in range(2):
            perm = np.concatenate([P * i + (np.arange(P) + 64 * c) % P
                                   for i in range(NKT)])
            qidx = np.concatenate([P * j + 64 * c + np.arange(64)
                                   for j in range(NKT)])
            dmask = np.zeros((NKT, P, 64), f32)
            for kc in range(NKT):
                gk = P * kc + (np.arange(P) + 64 * c) % P
                gq = P * kc + 64 * c + np.arange(64)
                dmask[kc] = np.where(tgt_mask[np.ix_(gq, gk)].T, NEG, 0.0)
            sa_padb = np.where(tgt_pad[b][perm], NEG, 0.0).astype(f32)
            ca_padb = np.where(src_pad[b], NEG, 0.0).astype(f32)
            cores.append(dict(
                b=b, c=c, qidx=qidx,
                in_map=dict(
                    tgt_rolled=np.ascontiguousarray(tgt[b][perm]),
                    tgt_q=np.ascontiguousarray(tgt[b][qidx] + sa_bo_eff[None, :]),
                    srcT=srcTb,
                    dmask=np.ascontiguousarray(dmask.transpose(1, 0, 2)),
                    sa_pad=np.ascontiguousarray(sa_padb.reshape(NKT, P).T),
                    ca_pad=np.ascontiguousarray(ca_padb.reshape(NKT, P).T),
                    sa_winT=prep["sa_winT"], sa_bq=prep["sa_bq"],
                    sa_woT=prep["sa_woT"],
                    ca_winT=prep["ca_winT"], ca_bq=prep["ca_bq"],
                    ca_woT=prep["ca_woT"], ca_bo=prep["ca_bo"],
                    onehot=prep["onehot"],
                ),
            ))
    return prep, cores


def kernel(**inputs):
    f32 = np.float32
    if "A" not in _cache:
        _cache["A"] = build_kernel_a()
    if "B" not in _cache:
        _cache["B"] = build_kernel_b()

    prep, cores = _host_prep(inputs)

    res_a = run_bass_kernel_spmd(_cache["A"], [c["in_map"] for c in cores],
                                 core_ids=list(range(8)))
    last_exec_ns["A"] = res_a.exec_time_ns

    # ---- host routing (f32: avoids bf16 argmax flips) ----
    all_x3 = np.concatenate([res_a.results[k]["xhat3"] for k in range(8)], 0)
    all_logits = all_x3 @ prep["router_wf"].T + prep["router_bf"]
    z = all_logits - all_logits.max(-1, keepdims=True)
    ez = np.exp(z)
    probs = ez / ez.sum(-1, keepdims=True)
    gate = probs.max(-1).astype(f32)
    idx = probs.argmax(-1)

    order = np.argsort(idx, kind="stable")
    counts = np.bincount(idx, minlength=E)
    assert counts.max() <= CAP, f"expert overflow: {counts}"
    starts = np.zeros(E + 1, np.int64)
    starts[1:] = np.cumsum(counts)

    xb = np.zeros((E, D, CAP), ml_dtypes.bfloat16)
    for e in range(E):
        toks = order[starts[e]:starts[e + 1]]
        xb[e, :, :len(toks)] = all_x3[toks].T

    in_maps_b = [dict(x3T=xb[e],
                      w1e=np.ascontiguousarray(prep["w1f"][e]),
                      b1e=np.ascontiguousarray(prep["b1c"][e]),
                      w2e=np.ascontiguousarray(prep["w2"][e]),
                      b2e=np.ascontiguousarray(prep["b2c"][e]))
                 for e in range(E)]
    res_b = run_bass_kernel_spmd(_cache["B"], in_maps_b, core_ids=list(range(8)))
    last_exec_ns["B"] = res_b.exec_time_ns

    # ---- host combine ----
    token_mask = np.asarray(inputs["token_mask"])
    tm = np.concatenate([token_mask[c["b"]][c["qidx"]] for c in cores])
    y_all = np.zeros((4096, D), f32)
    for e in range(E):
        toks = order[starts[e]:starts[e + 1]]
        y_all[toks] = res_b.results[e]["yT"][:, :len(toks)].T
    scale = (gate * tm.astype(f32))[:, None]

    out = np.zeros((B, T, D), f32)
    for k, c in enumerate(cores):
        sl = slice(k * 512, (k + 1) * 512)
        out[c["b"], c["qidx"]] = (res_a.results[k]["tgt2"]
                                  + scale[sl] * y_all[sl])
    return out
